# revision 30
# baseline (speedup 1.0000x reference)
"""Trainium2 Bass kernel for a 6-layer GPT-style transformer (B=8, T=500,
N=512, H=8, V=32000), data-parallel over batch across 8 NeuronCores.

kernel(**inputs) takes the full unsharded inputs and returns full logits
[B, T, V] float32.
"""

import sys

import numpy as np
import ml_dtypes

for _p in ("/opt/trn_rl_repo", "/root/.axon_site/_ro/trn_rl_repo"):
    if _p not in sys.path:
        sys.path.append(_p)

V, N, H, L, T, B = 32000, 512, 8, 6, 500, 8
HD = N // H          # 64
F = 4 * N            # 2048
P = 128
NT = 4               # token tiles
TS = [128, 128, 128, 116]
HALVES = [(0, 256), (244, 500)]
EPS = 1e-5
SCALE = float(N) ** -0.5
VW = 500             # head psum free width (64 * 500 = 32000)
VCH = 2000           # Wh streaming chunk width (16 chunks)

_BUILD_CACHE = {}


def _emit_ln(nc, tc, pools, xT, hbT, mybir, bass):
    """LayerNorm in transposed space: hbT (bf16) = (xT - mu) * rstd.

    xT: [128, 4, T] f32 sbuf, feature f = kk*128 + p on (p, kk); t on free.
    Stats (sum x, sum x^2) via ones-vector matmuls (reduce over partitions),
    then broadcast back over partitions with gpsimd.
    """
    sb, ps, const = pools["sb"], pools["psum"], pools["const"]
    f32, f32r = mybir.dt.float32, mybir.dt.float32r
    ones_r = const["ones_f32r"]

    sq = sb.tile([P, NT, T], f32r, tag="ln_sq", bufs=1)
    for _j in range(NT):
        nc.vector.tensor_mul(sq[:, _j, :], xT[:, _j, :], xT[:, _j, :])

    mu_b = sb.tile([P, T], f32, tag="ln_mub", bufs=2)
    rs_b = sb.tile([P, T], f32, tag="ln_rsb", bufs=2)
    cent = sq  # reuse
    # token-split pipeline: half B's scalar tail hides under half A's
    # downstream matmuls (ones vector is pre-scaled by 1/N).
    # All stats matmuls run before any apply pass (apply overwrites sq).
    sts = {}
    for (h0, h1) in HALVES:
        W = h1 - h0
        st0 = ps.tile([1, W], f32, tag="stat", bufs=4, name=f"st0_{h0}")
        st1 = ps.tile([1, W], f32, tag="stat", bufs=4, name=f"st1_{h0}")
        for kk in range(NT):
            nc.tensor.matmul(st0[:], lhsT=ones_r[:, 0:1],
                             rhs=xT[:, kk, h0:h1],
                             start=(kk == 0), stop=(kk == NT - 1))
        for kk in range(NT):
            nc.tensor.matmul(st1[:], lhsT=ones_r[:, 0:1],
                             rhs=sq[:, kk, h0:h1],
                             start=(kk == 0), stop=(kk == NT - 1))
        sts[h0] = (st0, st1)
    for hi, (h0, h1) in enumerate(HALVES):
        W = h1 - h0
        st0, st1 = sts[h0]
        # apply region is disjoint (no double-write -> no false deps on the
        # next consumers); stats tiles stay 256-wide for f32r full rate
        a0 = 0 if hi == 0 else HALVES[hi - 1][1]   # 0 / 256
        o = a0 - h0                                 # slice offset into stats
        AW = h1 - a0
        mu = sb.tile([1, W], f32, tag="ln_mu", bufs=2, name=f"mu_{h0}")
        var = sb.tile([1, W], f32, tag="ln_var", bufs=2, name=f"var_{h0}")
        tmp = sb.tile([1, W], f32, tag="ln_tmp", bufs=2, name=f"tmp_{h0}")
        scr = sb.tile([1, W], f32, tag="ln_scr", bufs=2, name=f"scr_{h0}")
        nc.scalar.copy(mu[:], st0[:])
        nc.scalar.activation(tmp[:], st0[:],
                             mybir.ActivationFunctionType.Square)
        nc.vector.tensor_tensor(var[:], st1[:], tmp[:],
                                op=mybir.AluOpType.subtract)
        nc.scalar.activation(var[:], var[:],
                             mybir.ActivationFunctionType.Sqrt,
                             bias=const["eps"][0:1, :])
        nc.vector.reciprocal_approx_fast(out=scr[:], in_=var[:])  # rstd
        nc.gpsimd.partition_broadcast(mu_b[:, a0:h1], mu[0:1, o:o + AW])
        nc.gpsimd.partition_broadcast(rs_b[:, a0:h1], scr[0:1, o:o + AW])
        for _j in range(NT):
            nc.vector.tensor_tensor(
                cent[:, _j, a0:h1], xT[:, _j, a0:h1],
                mu_b[:, None, a0:h1].to_broadcast([P, 1, AW]),
                op=mybir.AluOpType.subtract)
            nc.vector.tensor_tensor(
                hbT[:, _j, a0:h1], cent[:, _j, a0:h1],
                rs_b[:, None, a0:h1].to_broadcast([P, 1, AW]),
                op=mybir.AluOpType.mult)


def _build_program():
    import concourse.bass as bass
    import concourse.tile as tile
    from concourse import bacc, mybir
    from concourse.masks import make_identity

    f32 = mybir.dt.float32
    f32r = mybir.dt.float32r
    bf16 = mybir.dt.bfloat16
    i32 = mybir.dt.int32
    AF = mybir.ActivationFunctionType

    nc = bacc.Bacc("TRN2", target_bir_lowering=False, debug=False)

    idx_d = nc.dram_tensor("idx", [T, 1], i32, kind="ExternalInput")
    tok_d = nc.dram_tensor("tok", [V, N], f32, kind="ExternalInput")
    pos_d = nc.dram_tensor("pos", [T, N], f32, kind="ExternalInput")
    wq_d = nc.dram_tensor("wq", [L, P, NT, N], bf16, kind="ExternalInput")
    wk_d = nc.dram_tensor("wk", [L, P, NT, N], bf16, kind="ExternalInput")
    wv_d = nc.dram_tensor("wv", [L, P, NT, N], bf16, kind="ExternalInput")
    wo_d = nc.dram_tensor("wo", [L, P, NT, N], bf16, kind="ExternalInput")
    w1_d = nc.dram_tensor("w1", [L, P, NT, F], bf16, kind="ExternalInput")
    w2_d = nc.dram_tensor("w2", [L, P, 16, N], bf16, kind="ExternalInput")
    wh_d = nc.dram_tensor("wh", [P, NT, V], bf16, kind="ExternalInput")
    out_d = nc.dram_tensor("logits", [T, V], f32, kind="ExternalOutput")

    with tile.TileContext(nc) as tc:
        import contextlib
        with contextlib.ExitStack() as ctx:
            constp = ctx.enter_context(tc.tile_pool(name="const", bufs=1))
            sb = ctx.enter_context(tc.tile_pool(name="sb", bufs=1))
            persist = ctx.enter_context(tc.tile_pool(name="persist", bufs=1))
            wtp = ctx.enter_context(tc.tile_pool(name="wtp", bufs=1))
            psum = ctx.enter_context(tc.tile_pool(name="psum", bufs=1, space="PSUM"))
            psmm = psum

            # ---- constants ----
            ident_bf = constp.tile([P, P], bf16)
            make_identity(nc, ident_bf[:])
            ident_f = constp.tile([P, P], f32)
            make_identity(nc, ident_f[:])
            ones_f = constp.tile([P, 1], f32)
            nc.vector.memset(ones_f[:], 1.0 / N)
            ones_r = constp.tile([P, 1], f32r)
            nc.vector.tensor_copy(ones_r[:], ones_f[:])
            eps_t = constp.tile([P, 1], f32)
            nc.vector.memset(eps_t[:], EPS)
            # triu keep-mask: m[p, c] = 1 if p <= c else 0
            triu = constp.tile([P, P], bf16)
            nc.gpsimd.memset(triu[:], 1.0)
            nc.gpsimd.affine_select(
                out=triu[:], in_=triu[:],
                compare_op=mybir.AluOpType.is_ge, fill=0.0,
                base=0, pattern=[[1, P]], channel_multiplier=-1)
            const = {"ones_f32r": ones_r, "eps": eps_t}
            pools = {"sb": sb, "psum": psum, "const": const}

            dmo = constp.tile([1, 4], f32)

            def preload(func, anchor=None):
                # tiny ACT op pulls the table-set load of `func` into a
                # matmul-busy window; `anchor` (an AP) phase-orders it.
                # scale=0 + bias=eps makes the evaluated value safely positive.
                src_ap = anchor if anchor is not None else eps_t[0:1, 0:1]
                nc.scalar.activation(dmo[0:1, 0:1], src_ap, func,
                                     bias=eps_t[0:1, :], scale=0.0)

            preload(AF.Sqrt)

            # ---- embedding gather + pos, then transpose to xT ----
            xT = persist.tile([P, NT, T], f32r)
            x0 = sb.tile([P, NT, N], f32, tag="gt", bufs=1)
            posb = sb.tile([P, NT, N], f32, tag="ln_sq", bufs=1)
            for i in range(NT):
                idxt = sb.tile([P, 1], i32, tag="idx", bufs=2)
                nc.sync.dma_start(idxt[:TS[i]], idx_d[i * P:i * P + TS[i], :])
                nc.gpsimd.indirect_dma_start(
                    out=x0[:TS[i], i, :], out_offset=None,
                    in_=tok_d[:],
                    in_offset=bass.IndirectOffsetOnAxis(ap=idxt[:TS[i], :1], axis=0))
                nc.sync.dma_start(posb[:TS[i], i, :], pos_d[i * P:i * P + TS[i], :])
            for i in range(NT):
                nc.vector.tensor_add(x0[:TS[i], i, :], x0[:TS[i], i, :],
                                     posb[:TS[i], i, :])
            for i in range(NT):
                for kk in range(NT):
                    pt = psum.tile([P, P], f32, tag="mm", bufs=4)
                    nc.tensor.transpose(pt[:, :TS[i]], x0[:TS[i], i, bass.ts(kk, P)],
                                        ident_f[:TS[i], :TS[i]])
                    nc.vector.tensor_copy(xT[:, kk, i * P:i * P + TS[i]], pt[:, :TS[i]])

            # ---- transformer layers ----
            wpool_ctx = contextlib.ExitStack()
            wpool = wpool_ctx.enter_context(tc.tile_pool(name="wpool", bufs=2))
            for l in range(L):
                wq = wpool.tile([P, NT, N], bf16, tag="wq")
                wk = wpool.tile([P, NT, N], bf16, tag="wk")
                wv = wpool.tile([P, NT, N], bf16, tag="wv")
                wo = wpool.tile([P, NT, N], bf16, tag="wo")
                w1 = wpool.tile([P, NT, F], bf16, tag="w1")
                w2 = wpool.tile([P, 16, N], bf16, tag="w2")
                nc.sync.dma_start(wq[:], wq_d[l])
                nc.sync.dma_start(wk[:], wk_d[l])
                nc.sync.dma_start(wv[:], wv_d[l])
                nc.sync.dma_start(wo[:], wo_d[l])
                nc.sync.dma_start(w1[:], w1_d[l])
                nc.sync.dma_start(w2[:], w2_d[l])

                # LN1
                hbT = sb.tile([P, NT, T], bf16, tag="hbt", bufs=1)
                _emit_ln(nc, tc, pools, xT, hbT, mybir, bass)

                preload(AF.Exp, hbT[0:1, 0, 0:1])
                # Q^T, K^T  [P, NT, T] bf16
                QTb = sb.tile([P, NT, T], bf16, tag="qt", bufs=1)
                KTb = sb.tile([P, NT, T], bf16, tag="kt", bufs=1)
                for (h0, h1) in HALVES:
                    W = h1 - h0
                    for j in range(NT):
                        pq = psmm.tile([P, 512], f32, tag="mm", bufs=4)
                        for kk in range(NT):
                            nc.tensor.matmul(pq[:, :W],
                                             lhsT=wq[:, kk, bass.ts(j, P)],
                                             rhs=hbT[:, kk, h0:h1],
                                             start=(kk == 0), stop=(kk == NT - 1))
                        nc.vector.tensor_copy(QTb[:, j, h0:h1], pq[:, :W])
                    for j in range(NT):
                        pk = psmm.tile([P, 512], f32, tag="mm", bufs=4)
                        for kk in range(NT):
                            nc.tensor.matmul(pk[:, :W],
                                             lhsT=wk[:, kk, bass.ts(j, P)],
                                             rhs=hbT[:, kk, h0:h1],
                                             start=(kk == 0), stop=(kk == NT - 1))
                        nc.scalar.copy(KTb[:, j, h0:h1], pk[:, :W])

                # V rows, augmented with a ones column: Vaug[t, j, h, 0:64]=V,
                # [..., 64]=1  -> AV matmul also produces softmax denominators
                Vaug = sb.tile([P, NT, H, HD + 1], bf16, tag="vaug", bufs=1)
                nc.vector.memset(Vaug[:, :, :, HD:HD + 1], 1.0)
                for i in range(NT):
                    pv = psmm.tile([P, 512], f32, tag="mm", bufs=4)
                    for kk in range(NT):
                        nc.tensor.matmul(pv[:TS[i], :], lhsT=hbT[:, kk, i * P:i * P + TS[i]],
                                         rhs=wv[:, kk, :],
                                         start=(kk == 0), stop=(kk == NT - 1))
                    nc.vector.tensor_copy(
                        Vaug[:TS[i], i, :, 0:HD],
                        pv[:TS[i], :].rearrange("t (h d) -> t h d", h=H))

                # scores^T per (head, s-tile j): [s, t], t in [j*128, 500)
                # exp(scale * s) with no max-subtraction (|scores*scale| < 0.5),
                # then zero the not-yet-allowed (s > t) entries of the diagonal
                # block with a triangular 0/1 mask.
                wT = [wtp.tile([P, H, T - j * P], bf16, tag=f"wt{j}",
                               name=f"wt{j}_{l}") for j in range(NT)]
                for j in range(NT):
                    tr = T - j * P
                    for h in range(H):
                        pb = (h % 2) * 64
                        jj = h // 2
                        ps_ = psmm.tile([P, 512], f32, tag="mm", bufs=4)
                        nc.tensor.matmul(
                            ps_[:TS[j], :tr],
                            lhsT=KTb[pb:pb + HD, jj, j * P:j * P + TS[j]],
                            rhs=QTb[pb:pb + HD, jj, j * P:],
                            start=True, stop=True)
                        nc.scalar.activation(wT[j][:TS[j], h, :], ps_[:TS[j], :tr],
                                             AF.Exp, scale=SCALE)
                        nc.gpsimd.affine_select(
                            out=wT[j][:TS[j], h, 0:TS[j]],
                            in_=wT[j][:TS[j], h, 0:TS[j]],
                            compare_op=mybir.AluOpType.is_ge, fill=0.0,
                            base=0, pattern=[[1, TS[j]]], channel_multiplier=-1)

                preload(AF.Sqrt, wT[NT - 1][0:1, H - 1, 0:1])
                # AV (+ denominator) and normalization -> ab rows [t, N] bf16
                ab = sb.tile([P, NT, N], bf16, tag="ab", bufs=1)
                for i in range(NT):
                    zb = sb.tile([P, H], f32, tag="zb", bufs=2, name=f"zb{i}")
                    rz = sb.tile([P, H], f32, tag="rz", bufs=2, name=f"rz{i}")
                    for h in range(H):
                        pa = psum.tile([P, HD + 1], f32, tag="mm", bufs=4,
                                       name=f"pa{i}_{h}")
                        for j in range(i + 1):
                            nc.tensor.matmul(
                                pa[:TS[i], :],
                                lhsT=wT[j][:TS[j], h, (i - j) * P:(i - j) * P + TS[i]],
                                rhs=Vaug[:TS[j], j, h, :],
                                start=(j == 0), stop=(j == i))
                        nc.vector.tensor_copy(zb[:TS[i], h:h + 1],
                                              pa[:TS[i], HD:HD + 1])
                        nc.vector.tensor_copy(
                            ab[:TS[i], i, h * HD:(h + 1) * HD],
                            pa[:TS[i], 0:HD])
                    nc.vector.reciprocal_approx_fast(out=rz[:TS[i]],
                                                     in_=zb[:TS[i]])
                    for h in range(H):
                        nc.vector.tensor_scalar_mul(
                            ab[:TS[i], i, h * HD:(h + 1) * HD],
                            ab[:TS[i], i, h * HD:(h + 1) * HD],
                            rz[:TS[i], h:h + 1])

                # transpose ab -> aTb [d, t]
                aTb = sb.tile([P, NT, T], bf16, tag="at", bufs=1)
                for i in range(NT):
                    for kk in range(NT):
                        ptb = psum.tile([P, P], bf16, tag="mm", bufs=4)
                        nc.tensor.transpose(ptb[:, :TS[i]],
                                            ab[:TS[i], i, bass.ts(kk, P)],
                                            ident_bf[:TS[i], :TS[i]])
                        nc.vector.tensor_copy(aTb[:, kk, i * P:i * P + TS[i]],
                                              ptb[:, :TS[i]])

                # out proj (transposed) + residual
                for j in range(NT):
                    po = psmm.tile([P, 512], f32, tag="mm", bufs=4)
                    for kk in range(NT):
                        nc.tensor.matmul(po[:, :T], lhsT=wo[:, kk, bass.ts(j, P)],
                                         rhs=aTb[:, kk, :],
                                         start=(kk == 0), stop=(kk == NT - 1))
                    nc.vector.tensor_add(xT[:, j, :], xT[:, j, :], po[:, :T])

                # LN2 + MLP
                h2T = sb.tile([P, NT, T], bf16, tag="hbt", bufs=1)
                _emit_ln(nc, tc, pools, xT, h2T, mybir, bass)
                preload(AF.Gelu, h2T[0:1, 0, 0:1])
                gT = sb.tile([P, 16, T], bf16, tag="gt", bufs=1)
                for (h0, h1) in HALVES:
                    W = h1 - h0
                    for jj in range(16):
                        pg = psmm.tile([P, 512], f32, tag="mm", bufs=4)
                        for kk in range(NT):
                            nc.tensor.matmul(pg[:, :W],
                                             lhsT=w1[:, kk, bass.ts(jj, P)],
                                             rhs=h2T[:, kk, h0:h1],
                                             start=(kk == 0), stop=(kk == NT - 1))
                        nc.scalar.activation(gT[:, jj, h0:h1], pg[:, :W], AF.Gelu)
                preload(AF.Sqrt, gT[0:1, 15, T - 1:T])
                for j in range(NT):
                    pm = psmm.tile([P, 512], f32, tag="mm", bufs=4)
                    for kk in range(16):
                        nc.tensor.matmul(pm[:, :T], lhsT=w2[:, kk, bass.ts(j, P)],
                                         rhs=gT[:, kk, :],
                                         start=(kk == 0), stop=(kk == 15))
                    nc.vector.tensor_add(xT[:, j, :], xT[:, j, :], pm[:, :T])

            # ---- final LN + head ----
            hfT = sb.tile([P, NT, T], bf16, tag="hbt", bufs=1)
            _emit_ln(nc, tc, pools, xT, hfT, mybir, bass)
            wpool_ctx.close()
            whp = ctx.enter_context(tc.tile_pool(name="whp", bufs=2))
            NCH = V // VCH
            wh_tiles = {}

            def load_wh(c):
                t_ = whp.tile([P, NT, VCH], bf16, tag="wh", name=f"wh{c}",
                              bufs=2)
                nc.gpsimd.dma_start(t_[:], wh_d[:, :, c * VCH:(c + 1) * VCH])
                wh_tiles[c] = t_

            load_wh(0)
            for c in range(NCH):
                if c + 1 < NCH:
                    load_wh(c + 1)
                whc = wh_tiles.pop(c)
                for i in range(NT):
                    stg = whp.tile([P, VCH], f32, tag="lg", bufs=3,
                                   name=f"stg{c}_{i}")
                    for vv in range(VCH // VW):
                        ph = psmm.tile([P, 512], f32, tag="mm", bufs=4)
                        for kk in range(NT):
                            nc.tensor.matmul(
                                ph[:TS[i], :VW],
                                lhsT=hfT[:, kk, i * P:i * P + TS[i]],
                                rhs=whc[:, kk, vv * VW:(vv + 1) * VW],
                                start=(kk == 0), stop=(kk == NT - 1))
                        if vv % 2 == 0:
                            nc.vector.tensor_copy(
                                stg[:TS[i], vv * VW:(vv + 1) * VW],
                                ph[:TS[i], :VW])
                        else:
                            nc.scalar.copy(
                                stg[:TS[i], vv * VW:(vv + 1) * VW],
                                ph[:TS[i], :VW])
                    QC = VCH // 4
                    for qq in range(4):
                        nc.sync.dma_start(
                            out_d[i * P:i * P + TS[i],
                                  c * VCH + qq * QC:c * VCH + (qq + 1) * QC],
                            stg[:TS[i], qq * QC:(qq + 1) * QC])

    nc.compile()
    return nc


def _get_program():
    if "nc" not in _BUILD_CACHE:
        _BUILD_CACHE["nc"] = _build_program()
    return _BUILD_CACHE["nc"]


def _prep_inputs(idx, tok_emb, pos_emb, Wq, Wk, Wv, Wo, ln1_g, ln2_g, lnf_g,
                 W1, W2, Wh):
    """Host-side prep: per-core input dicts (fold LN gains into the following
    weight matrices, cast weights to bf16, relayout to [P, ksub, ...])."""
    bf = ml_dtypes.bfloat16

    def kpart(w):  # [K, M] -> [P, K//P, M]
        k, m = w.shape[-2], w.shape[-1]
        return np.ascontiguousarray(
            w.reshape(w.shape[:-2] + (k // P, P, m)).swapaxes(-3, -2))

    g1 = ln1_g[:, None, :].astype(np.float32)        # [L, 1, N]
    g2 = ln2_g[:, None, :].astype(np.float32)
    wq = kpart((Wq * g1.transpose(0, 2, 1)).astype(bf))
    wk = kpart((Wk * g1.transpose(0, 2, 1)).astype(bf))
    wv = kpart((Wv * g1.transpose(0, 2, 1)).astype(bf))
    wo = kpart(Wo.astype(bf))
    w1 = kpart((W1 * g2.transpose(0, 2, 1)).astype(bf))
    w2 = kpart(W2.astype(bf))
    wh = kpart((Wh * lnf_g[:, None].astype(np.float32)).astype(bf))

    shared = dict(
        tok=np.ascontiguousarray(tok_emb.astype(np.float32)),
        pos=np.ascontiguousarray(pos_emb[:T].astype(np.float32)),
        wq=wq, wk=wk, wv=wv, wo=wo, w1=w1, w2=w2, wh=wh)
    in_maps = []
    for c in range(B):
        m = dict(shared)
        m["idx"] = np.ascontiguousarray(idx[c].astype(np.int32).reshape(T, 1))
        in_maps.append(m)
    return in_maps


def run(inputs, trace=False):
    from concourse.bass_utils import run_bass_kernel_spmd

    in_maps = _prep_inputs(
        inputs["idx"], inputs["tok_emb"], inputs["pos_emb"], inputs["Wq"],
        inputs["Wk"], inputs["Wv"], inputs["Wo"], inputs["ln1_g"],
        inputs["ln2_g"], inputs["lnf_g"], inputs["W1"], inputs["W2"],
        inputs["Wh"])
    nc = _get_program()
    res = run_bass_kernel_spmd(nc, in_maps, core_ids=list(range(B)),
                               trace=trace)
    logits = np.stack([res.results[c]["logits"] for c in range(B)], axis=0)
    return logits.astype(np.float32), res


def kernel(**inputs):
    logits, _ = run(inputs, trace=False)
    return logits
